# revision 36
# baseline (speedup 1.0000x reference)
"""Trainium2 Bass kernel for nn_BiaffineModule (biaffine span scorer).

Math (reference):
    x  = concat(final_hidden, feature_vecs)        [B,S,H+F]
    s  = x @ start_W + start_b                     [B,S,T]
    e  = x @ end_W + end_b                         [B,S,T]
    pre[b,s,e,c]  = sum_u (s @ U[:,c,:])[b,s,u] * e[b,e,u]
    ffn[b,s,e,c]  = (s@Ws)[b,s,c] + (e@We)[b,e,c] + (wh@Ww)[s,e,c] + lin_b[c]
    out = pre + ffn                                [B,S,S,C]

Sharding: the start axis `s` is split 8 ways (32 rows per core); each core
computes the full [B, 32, S, C] slab of the pairwise grid for all batches.
Small params are replicated; width_hidden is sliced per core.

All large operands are fp16 (PSUM accumulation is fp32): rel err vs the
fp32 reference is ~5e-4, well inside the 2e-2 gate, DMA bytes halve vs
fp32, and every matmul runs at 1 cycle/row (no f32r small-N penalty).

Per-core dataflow (contraction dims live on SBUF partitions; the host
pre-transposes inputs so the device never transposes):
    sT   [256,128]   = sW_aug^T @ xT-slab-cols
    fsT  [16,128]    = Ws^T @ sT + lin_b (K=1 ones fold)
    sUT  [u, c*128+row] = sum_t U[t,c,u] sT[t,row]
    fw   [e,(c,s)]   = sum_w whT[w,e] Ww[w,c]      (per-s stationary wh)
    eT   [256,256] per b = eW_aug^T @ xT[b]
    out[e,(c,s)] per (b,e-chunk), PSUM-accumulated:
        2 MMs biaffine (eT x sUT) + 1 K=1 MM fs fold,
    then one DVE add of the (fw+fe) plane and a contiguous store.

Host unshards results[k][b,e,(c,s)] -> full[b, k*32+s, e, c].
"""

import sys

import numpy as np

sys.path.insert(0, "/opt/trn_rl_repo")

B, S, H, F = 4, 256, 768, 32
T, WD, C = 256, 64, 16
NCORES = 8
SLAB = S // NCORES          # 32 s-rows per core
ROWS = B * SLAB             # 128 slab rows (b-major, s-minor)
NB = B * S                  # 1024 xT columns (b-major, s-minor)
KPAD = 896                  # 7 * 128 (zero-padded H+F+1 ones-row)
NKCH = KPAD // 128

_CACHE = {}


def _build():
    import concourse.bacc as bacc
    import concourse.mybir as mybir
    from concourse import tile

    f32 = mybir.dt.float32
    f16 = mybir.dt.float16

    nc = bacc.Bacc(
        "TRN2", target_bir_lowering=False, debug=False, num_devices=NCORES
    )

    # fp16 inputs, host-transposed so contraction dims sit on partitions
    sxw_d = nc.dram_tensor("sxw", [KPAD, T + ROWS], f16, kind="ExternalInput")
    eW_d = nc.dram_tensor("eW", [KPAD, T], f16, kind="ExternalInput")
    xT_d = nc.dram_tensor("xT", [KPAD, NB], f16, kind="ExternalInput")
    U_d = nc.dram_tensor("U2", [T, C * T], f16, kind="ExternalInput")
    wh_d = nc.dram_tensor("whT2", [WD, SLAB * S], f16, kind="ExternalInput")
    lp_d = nc.dram_tensor("linPack", [128, 5 * C], f16, kind="ExternalInput")
    lw_d = nc.dram_tensor("linWw", [WD, C], f16, kind="ExternalInput")
    out_d = nc.dram_tensor("out", [B, S, SLAB * C], f32, kind="ExternalOutput")

    with tile.TileContext(nc) as tc:
        with (
            tc.tile_pool(name="consts", bufs=1) as consts,
            tc.tile_pool(name="acts", bufs=1) as acts,
            tc.tile_pool(name="outp", bufs=3) as outp,
            tc.tile_pool(name="pmm", bufs=2, space="PSUM") as pmm,
            tc.tile_pool(name="pmm2", bufs=2, space="PSUM") as pmm2,
            tc.tile_pool(name="ps5", bufs=3, space="PSUM") as ps5,
            tc.tile_pool(name="pfw", bufs=1, space="PSUM") as pfw,
        ):
            ones = consts.tile([1, 128], f16)
            nc.vector.memset(ones[:], 1.0)

            # ---- loads: sync ring in compute-consumption order; small params
            # and all stores/gathers ride the scalar ring.
            sxwb = consts.tile([128, NKCH, T + ROWS], f16)
            sxv = sxw_d.ap().rearrange("(n p) w -> p n w", p=128)
            nc.sync.dma_start(sxwb[:, 0:4, :], sxv[:, 0:4, :])
            nc.sync.dma_start(sxwb[:, 4:NKCH, :], sxv[:, 4:NKCH, :])
            lpb = consts.tile([128, 5, C], f16)
            nc.scalar.dma_start(lpb[:], lp_d.ap().rearrange("p (n c) -> p n c", c=C))
            lwb = consts.tile([WD, C], f16)
            nc.scalar.dma_start(lwb[:], lw_d[:])

            Ub = consts.tile([128, 2, C * T], f16)
            Uv = U_d.ap().rearrange("(n p) w -> p n w", p=128)
            nc.sync.dma_start(Ub[:, :, 0 : 4 * T], Uv[:, :, 0 : 4 * T])
            nc.sync.dma_start(Ub[:, :, 4 * T : 8 * T], Uv[:, :, 4 * T : 8 * T])
            whb = consts.tile([WD, SLAB * S], f16)
            nc.sync.dma_start(whb[:], wh_d[:])
            nc.sync.dma_start(Ub[:, :, 8 * T : 12 * T], Uv[:, :, 8 * T : 12 * T])
            nc.sync.dma_start(Ub[:, :, 12 * T : 16 * T], Uv[:, :, 12 * T : 16 * T])
            eWb = consts.tile([128, NKCH, T], f16)
            nc.sync.dma_start(eWb[:], eW_d.ap().rearrange("(n p) w -> p n w", p=128))
            xTb = consts.tile([128, NKCH, NB], f16)
            xv = xT_d.ap().rearrange("(n p) w -> p n w", p=128)
            for b in range(B):
                nc.sync.dma_start(
                    xTb[:, :, b * S : (b + 1) * S], xv[:, :, b * S : (b + 1) * S]
                )

            # ---- sT [t, slab-row] ----------------------------------------------
            sT = acts.tile([128, 2, ROWS], f16)
            for tch in range(2):
                ps = pmm.tile([128, ROWS], f32, tag="pmm")
                for k in range(NKCH):
                    nc.tensor.matmul(
                        ps[:],
                        sxwb[:, k, tch * 128 : (tch + 1) * 128],
                        sxwb[:, k, T : T + ROWS],
                        start=(k == 0),
                        stop=(k == NKCH - 1),
                    )
                nc.vector.tensor_copy(sT[:, tch, :], ps[:])

            # ---- fsT [c, slab-row] = Ws^T @ sT + lin_b -------------------------
            fsT = acts.tile([16, ROWS], f16)
            fsz = acts.tile([1, B, SLAB * C], f16)
            psf = pmm.tile([16, ROWS], f32, tag="pmm")
            for tch in range(2):
                nc.tensor.matmul(
                    psf[:], lpb[:, tch, :], sT[:, tch, :],
                    start=(tch == 0), stop=False,
                )
            nc.tensor.matmul(psf[:], lpb[0:1, 4, :], ones[:], start=False, stop=True)
            nc.vector.tensor_copy(fsT[:], psf[:])
            # fs row per b -> partition 0, (c,s) order
            for b in range(B):
                nc.scalar.dma_start(fsz[0:1, b, :], fsT[:, b * SLAB : (b + 1) * SLAB])

            # ---- sUT [u, (c, slab-row)] per c-group ----------------------------
            sUT = [acts.tile([128, C, ROWS], f16, name=f"sUT{u}") for u in range(2)]

            def sUT_group(grp):
                for uch in range(2):
                    ps = pmm.tile([128, 512], f32, tag="pmm")
                    for cl in range(4):
                        c = grp * 4 + cl
                        for tch in range(2):
                            nc.tensor.matmul(
                                ps[:, cl * 128 : (cl + 1) * 128],
                                Ub[:, tch, c * T + uch * 128 : c * T + uch * 128 + 128],
                                sT[:, tch, :],
                                start=(tch == 0),
                                stop=(tch == 1),
                            )
                    nc.vector.tensor_copy(
                        sUT[uch][:, grp * 4 : (grp + 1) * 4, :], ps[:]
                    )

            # ---- fw plane [e, (c,s)] per e-chunk -------------------------------
            fw = acts.tile([128, 2, C, SLAB], f32)

            def fw_stage(ech):
                ps = pfw.tile([128, SLAB, C], f32, tag="pfw")
                for s in range(SLAB):
                    nc.tensor.matmul(
                        ps[:, s, :],
                        whb[:, s * S + ech * 128 : s * S + ech * 128 + 128],
                        lwb[:],
                        start=True,
                        stop=True,
                    )
                src = ps[:]
                src = type(src)(
                    src.tensor, src.offset, [src.ap[0], [1, C], [C, SLAB]]
                )
                nc.vector.tensor_copy(fw[:, ech, :, :], src)

            # ---- per-batch: eT, fe planes, biaffine + folds, store -------------
            eT = acts.tile([128, 2, NB], f16)
            planes = acts.tile([128, 2, SLAB * C], f32, name="planes")

            def eT_half(h):
                for tch in range(2):
                    ps = pmm2.tile([128, 2 * S], f32, tag="pmm2")
                    for k in range(NKCH):
                        nc.tensor.matmul(
                            ps[:],
                            eWb[:, k, tch * 128 : (tch + 1) * 128],
                            xTb[:, k, h * 2 * S : (h + 1) * 2 * S],
                            start=(k == 0),
                            stop=(k == NKCH - 1),
                        )
                    nc.vector.tensor_copy(
                        eT[:, tch, h * 2 * S : (h + 1) * 2 * S], ps[:]
                    )

            def batch_stage(b):
                for ech in range(2):
                    ecols = slice(b * S + ech * 128, b * S + ech * 128 + 128)
                    psq = pmm.tile([128, 16], f32, tag="pmm", name=f"feq{b}{ech}")
                    for tch in range(2):
                        nc.tensor.matmul(
                            psq[:],
                            eT[:, tch, ecols],
                            lpb[:, 2 + tch, :],
                            start=(tch == 0),
                            stop=(tch == 1),
                        )
                    feb = psq[:]
                    feb = type(feb)(feb.tensor, feb.offset, [feb.ap[0], [1, C], [0, SLAB]])
                    nc.vector.tensor_add(
                        planes[:, ech, :].rearrange("p (c s) -> p c s", c=C),
                        fw[:, ech, :, :],
                        feb,
                    )

                for ech in range(2):
                    ps = ps5.tile([128, SLAB * C], f32, tag="ps5")
                    ecols = slice(b * S + ech * 128, b * S + ech * 128 + 128)
                    for uch in range(2):
                        nc.tensor.matmul(
                            ps[:],
                            eT[:, uch, ecols],
                            sUT[uch][:, :, b * SLAB : (b + 1) * SLAB],
                            start=(uch == 0),
                            stop=False,
                        )
                    nc.tensor.matmul(
                        ps[:], ones[:], fsz[0:1, b, :], start=False, stop=True
                    )
                    ob = outp.tile([128, SLAB * C], f32, tag="outp")
                    nc.vector.tensor_add(ob[:], ps[:], planes[:, ech, :])
                    nc.scalar.dma_start(
                        out_d[b, ech * 128 : (ech + 1) * 128, :], ob[:]
                    )

            # emission order matched to DMA arrival order above
            sUT_group(0)
            sUT_group(1)
            fw_stage(0)
            fw_stage(1)
            sUT_group(2)
            sUT_group(3)
            eT_half(0)
            batch_stage(0)
            batch_stage(1)
            eT_half(1)
            batch_stage(2)
            batch_stage(3)

    nc.compile()
    return nc


def _get_nc():
    if "nc" not in _CACHE:
        _CACHE["nc"] = _build()
    return _CACHE["nc"]


def kernel(
    final_hidden, feature_vecs, start_W, start_b, end_W, end_b, U,
    width_hidden, lin_W, lin_b,
):
    from concourse.bass_utils import run_bass_kernel_spmd

    f32 = np.float32
    f16 = np.float16
    fh = np.asarray(final_hidden, f32)
    fv = np.asarray(feature_vecs, f32)

    x = np.concatenate([fh, fv], axis=-1)                  # [B,S,H+F]
    xT = np.zeros((KPAD, NB), f32)
    xT[: H + F] = x.reshape(NB, H + F).T
    xT[H + F] = 1.0                                        # bias fold row
    xT16 = xT.astype(f16)

    def aug(W, bvec):
        Wa = np.zeros((KPAD, T), f32)
        Wa[: H + F] = np.asarray(W, f32)
        Wa[H + F] = np.asarray(bvec, f32)
        return Wa.astype(f16)

    sW = aug(start_W, start_b)
    eW = aug(end_W, end_b)
    U2 = np.ascontiguousarray(np.asarray(U, f32).reshape(T, C * T)).astype(f16)
    linW = np.asarray(lin_W, f32)
    linWw = np.ascontiguousarray(linW[2 * T :]).astype(f16)
    linPack = np.zeros((128, 5 * C), f32)
    linPack[:, 0:C] = linW[0:128, :]
    linPack[:, C : 2 * C] = linW[128:256, :]
    linPack[:, 2 * C : 3 * C] = linW[T : T + 128, :]
    linPack[:, 3 * C : 4 * C] = linW[T + 128 : 2 * T, :]
    linPack[0, 4 * C : 5 * C] = np.asarray(lin_b, f32)
    linPack16 = linPack.astype(f16)
    wh = np.asarray(width_hidden, f32)

    in_maps = []
    for k in range(NCORES):
        slab = wh[k * SLAB : (k + 1) * SLAB]               # [32, 256, 64]
        whT2 = np.ascontiguousarray(
            slab.transpose(2, 0, 1).reshape(WD, SLAB * S)
        ).astype(f16)
        cols = (
            np.arange(B)[:, None] * S + (k * SLAB + np.arange(SLAB))[None, :]
        ).reshape(-1)
        sxw = np.ascontiguousarray(np.concatenate([sW, xT16[:, cols]], axis=1))
        in_maps.append(
            {
                "sxw": sxw, "eW": eW, "xT": xT16, "U2": U2,
                "whT2": whT2, "linPack": linPack16, "linWw": linWw,
            }
        )

    _CACHE["last_in_maps"] = in_maps
    nc = _get_nc()
    res = run_bass_kernel_spmd(nc, in_maps, core_ids=list(range(NCORES)))

    full = np.empty((B, S, S, C), f32)
    for k in range(NCORES):
        r = res.results[k]["out"].reshape(B, S, C, SLAB)
        full[:, k * SLAB : (k + 1) * SLAB] = r.transpose(0, 3, 1, 2)
    return full


# revision 37
# speedup vs baseline: 1.0153x; 1.0153x over previous
"""Trainium2 Bass kernel for nn_BiaffineModule (biaffine span scorer).

Math (reference):
    x  = concat(final_hidden, feature_vecs)        [B,S,H+F]
    s  = x @ start_W + start_b                     [B,S,T]
    e  = x @ end_W + end_b                         [B,S,T]
    pre[b,s,e,c]  = sum_u (s @ U[:,c,:])[b,s,u] * e[b,e,u]
    ffn[b,s,e,c]  = (s@Ws)[b,s,c] + (e@We)[b,e,c] + (wh@Ww)[s,e,c] + lin_b[c]
    out = pre + ffn                                [B,S,S,C]

Sharding: the start axis `s` is split 8 ways (32 rows per core); each core
computes the full [B, 32, S, C] slab of the pairwise grid for all batches.
Small params are replicated; width_hidden is sliced per core.

All large operands are fp16 (PSUM accumulation is fp32): rel err vs the
fp32 reference is ~5e-4, well inside the 2e-2 gate, DMA bytes halve vs
fp32, and every matmul runs at 1 cycle/row (no f32r small-N penalty).

Per-core dataflow (contraction dims live on SBUF partitions; the host
pre-transposes inputs so the device never transposes):
    sT   [256,128]   = sW_aug^T @ xT-slab-cols
    fsT  [16,128]    = Ws^T @ sT + lin_b (K=1 ones fold)
    sUT  [u, c*128+row] = sum_t U[t,c,u] sT[t,row]
    fw   [e,(c,s)]   = sum_w whT[w,e] Ww[w,c]      (per-s stationary wh)
    eT   [256,256] per b = eW_aug^T @ xT[b]
    out[e,(c,s)] per (b,e-chunk), PSUM-accumulated:
        2 MMs biaffine (eT x sUT) + 1 K=1 MM fs fold,
    then one DVE add of the (fw+fe) plane and a contiguous store.

Host unshards results[k][b,e,(c,s)] -> full[b, k*32+s, e, c].
"""

import sys

import numpy as np

sys.path.insert(0, "/opt/trn_rl_repo")

B, S, H, F = 4, 256, 768, 32
T, WD, C = 256, 64, 16
NCORES = 8
SLAB = S // NCORES          # 32 s-rows per core
ROWS = B * SLAB             # 128 slab rows (b-major, s-minor)
NB = B * S                  # 1024 xT columns (b-major, s-minor)
KPAD = 896                  # 7 * 128 (zero-padded H+F+1 ones-row)
NKCH = KPAD // 128

_CACHE = {}


def _build():
    import concourse.bacc as bacc
    import concourse.mybir as mybir
    from concourse import tile

    f32 = mybir.dt.float32
    f16 = mybir.dt.float16

    nc = bacc.Bacc(
        "TRN2", target_bir_lowering=False, debug=False, num_devices=NCORES
    )

    # fp16 inputs, host-transposed so contraction dims sit on partitions
    sxw_d = nc.dram_tensor("sxw", [KPAD, T + ROWS], f16, kind="ExternalInput")
    eW_d = nc.dram_tensor("eW", [KPAD, T], f16, kind="ExternalInput")
    xT_d = nc.dram_tensor("xT", [KPAD, NB], f16, kind="ExternalInput")
    U_d = nc.dram_tensor("U2", [T, C * T], f16, kind="ExternalInput")
    wh_d = nc.dram_tensor("whT2", [WD, SLAB * S], f16, kind="ExternalInput")
    lp_d = nc.dram_tensor("linPack", [128, 5 * C], f16, kind="ExternalInput")
    lw_d = nc.dram_tensor("linWw", [WD, C], f16, kind="ExternalInput")
    out_d = nc.dram_tensor("out", [B, S, SLAB * C], f32, kind="ExternalOutput")

    with tile.TileContext(nc) as tc:
        with (
            tc.tile_pool(name="consts", bufs=1) as consts,
            tc.tile_pool(name="acts", bufs=1) as acts,
            tc.tile_pool(name="outp", bufs=3) as outp,
            tc.tile_pool(name="pmm", bufs=2, space="PSUM") as pmm,
            tc.tile_pool(name="pmm2", bufs=2, space="PSUM") as pmm2,
            tc.tile_pool(name="ps5", bufs=3, space="PSUM") as ps5,
            tc.tile_pool(name="pfw", bufs=1, space="PSUM") as pfw,
        ):
            ones = consts.tile([1, 128], f16)
            nc.vector.memset(ones[:], 1.0)

            # ---- loads: sync ring in compute-consumption order; small params
            # and all stores/gathers ride the scalar ring.
            sxwb = consts.tile([128, NKCH, T + ROWS], f16)
            sxv = sxw_d.ap().rearrange("(n p) w -> p n w", p=128)
            nc.sync.dma_start(sxwb[:, 0:4, :], sxv[:, 0:4, :])
            nc.sync.dma_start(sxwb[:, 4:NKCH, :], sxv[:, 4:NKCH, :])
            lpb = consts.tile([128, 5, C], f16)
            nc.scalar.dma_start(lpb[:], lp_d.ap().rearrange("p (n c) -> p n c", c=C))
            lwb = consts.tile([WD, C], f16)
            nc.scalar.dma_start(lwb[:], lw_d[:])

            Ub = consts.tile([128, 2, C * T], f16)
            Uv = U_d.ap().rearrange("(n p) w -> p n w", p=128)
            nc.sync.dma_start(Ub[:, :, 0 : 4 * T], Uv[:, :, 0 : 4 * T])
            nc.sync.dma_start(Ub[:, :, 4 * T : 8 * T], Uv[:, :, 4 * T : 8 * T])
            whb = consts.tile([WD, SLAB * S], f16)
            nc.sync.dma_start(whb[:], wh_d[:])
            nc.sync.dma_start(Ub[:, :, 8 * T : 12 * T], Uv[:, :, 8 * T : 12 * T])
            nc.sync.dma_start(Ub[:, :, 12 * T : 16 * T], Uv[:, :, 12 * T : 16 * T])
            eWb = consts.tile([128, NKCH, T], f16)
            nc.sync.dma_start(eWb[:], eW_d.ap().rearrange("(n p) w -> p n w", p=128))
            xTb = consts.tile([128, NKCH, NB], f16)
            xv = xT_d.ap().rearrange("(n p) w -> p n w", p=128)
            for b in range(B):
                nc.sync.dma_start(
                    xTb[:, :, b * S : (b + 1) * S], xv[:, :, b * S : (b + 1) * S]
                )

            # ---- sT [t, slab-row] ----------------------------------------------
            sT = acts.tile([128, 2, ROWS], f16)
            for tch in range(2):
                ps = pmm.tile([128, ROWS], f32, tag="pmm")
                for k in range(NKCH):
                    nc.tensor.matmul(
                        ps[:],
                        sxwb[:, k, tch * 128 : (tch + 1) * 128],
                        sxwb[:, k, T : T + ROWS],
                        start=(k == 0),
                        stop=(k == NKCH - 1),
                    )
                nc.vector.tensor_copy(sT[:, tch, :], ps[:])

            # ---- fsT [c, slab-row] = Ws^T @ sT + lin_b -------------------------
            fsT = acts.tile([16, ROWS], f16)
            fsz = acts.tile([1, B, SLAB * C], f16)
            psf = pmm.tile([16, ROWS], f32, tag="pmm")
            for tch in range(2):
                nc.tensor.matmul(
                    psf[:], lpb[:, tch, :], sT[:, tch, :],
                    start=(tch == 0), stop=False,
                )
            nc.tensor.matmul(psf[:], lpb[0:1, 4, :], ones[:], start=False, stop=True)
            nc.vector.tensor_copy(fsT[:], psf[:])
            # fs row per b -> partition 0, (c,s) order
            for b in range(B):
                nc.scalar.dma_start(fsz[0:1, b, :], fsT[:, b * SLAB : (b + 1) * SLAB])

            # ---- sUT [u, (c, slab-row)] per c-group ----------------------------
            sUT = [acts.tile([128, C, ROWS], f16, name=f"sUT{u}") for u in range(2)]

            def sUT_group(grp):
                for uch in range(2):
                    ps = pmm.tile([128, 512], f32, tag="pmm")
                    for cl in range(4):
                        c = grp * 4 + cl
                        for tch in range(2):
                            nc.tensor.matmul(
                                ps[:, cl * 128 : (cl + 1) * 128],
                                Ub[:, tch, c * T + uch * 128 : c * T + uch * 128 + 128],
                                sT[:, tch, :],
                                start=(tch == 0),
                                stop=(tch == 1),
                            )
                    nc.vector.tensor_copy(
                        sUT[uch][:, grp * 4 : (grp + 1) * 4, :], ps[:]
                    )

            # ---- fw plane [e, (c,s)] per e-chunk -------------------------------
            fw = acts.tile([128, 2, C, SLAB], f32)

            def fw_stage(ech):
                ps = pfw.tile([128, SLAB, C], f32, tag="pfw")
                for s in range(SLAB):
                    nc.tensor.matmul(
                        ps[:, s, :],
                        whb[:, s * S + ech * 128 : s * S + ech * 128 + 128],
                        lwb[:],
                        start=True,
                        stop=True,
                    )
                src = ps[:]
                src = type(src)(
                    src.tensor, src.offset, [src.ap[0], [1, C], [C, SLAB]]
                )
                nc.vector.tensor_copy(fw[:, ech, :, :], src)

            # ---- per-batch: eT, fe planes, biaffine + folds, store -------------
            eT = acts.tile([128, 2, NB], f16)
            planes = acts.tile([128, 2, SLAB * C], f32, name="planes")

            def batch_stage(b):
                for tch in range(2):
                    ps = pmm2.tile([128, S], f32, tag="pmm2")
                    for k in range(NKCH):
                        nc.tensor.matmul(
                            ps[:],
                            eWb[:, k, tch * 128 : (tch + 1) * 128],
                            xTb[:, k, b * S : (b + 1) * S],
                            start=(k == 0),
                            stop=(k == NKCH - 1),
                        )
                    nc.vector.tensor_copy(eT[:, tch, b * S : (b + 1) * S], ps[:])

                for ech in range(2):
                    ecols = slice(b * S + ech * 128, b * S + ech * 128 + 128)
                    psq = pmm.tile([128, 16], f32, tag="pmm", name=f"feq{b}{ech}")
                    for tch in range(2):
                        nc.tensor.matmul(
                            psq[:],
                            eT[:, tch, ecols],
                            lpb[:, 2 + tch, :],
                            start=(tch == 0),
                            stop=(tch == 1),
                        )
                    feb = psq[:]
                    feb = type(feb)(feb.tensor, feb.offset, [feb.ap[0], [1, C], [0, SLAB]])
                    nc.vector.tensor_add(
                        planes[:, ech, :].rearrange("p (c s) -> p c s", c=C),
                        fw[:, ech, :, :],
                        feb,
                    )

                for ech in range(2):
                    ps = ps5.tile([128, SLAB * C], f32, tag="ps5")
                    ecols = slice(b * S + ech * 128, b * S + ech * 128 + 128)
                    for uch in range(2):
                        nc.tensor.matmul(
                            ps[:],
                            eT[:, uch, ecols],
                            sUT[uch][:, :, b * SLAB : (b + 1) * SLAB],
                            start=(uch == 0),
                            stop=False,
                        )
                    nc.tensor.matmul(
                        ps[:], ones[:], fsz[0:1, b, :], start=False, stop=True
                    )
                    ob = outp.tile([128, SLAB * C], f32, tag="outp")
                    nc.vector.tensor_add(ob[:], ps[:], planes[:, ech, :])
                    nc.scalar.dma_start(
                        out_d[b, ech * 128 : (ech + 1) * 128, :], ob[:]
                    )

            # emission order matched to DMA arrival order above
            sUT_group(0)
            sUT_group(1)
            fw_stage(0)
            fw_stage(1)
            sUT_group(2)
            sUT_group(3)
            for b in range(B):
                batch_stage(b)

    nc.compile()
    return nc


def _get_nc():
    if "nc" not in _CACHE:
        _CACHE["nc"] = _build()
    return _CACHE["nc"]


def kernel(
    final_hidden, feature_vecs, start_W, start_b, end_W, end_b, U,
    width_hidden, lin_W, lin_b,
):
    from concourse.bass_utils import run_bass_kernel_spmd

    f32 = np.float32
    f16 = np.float16
    fh = np.asarray(final_hidden, f32)
    fv = np.asarray(feature_vecs, f32)

    x = np.concatenate([fh, fv], axis=-1)                  # [B,S,H+F]
    xT = np.zeros((KPAD, NB), f32)
    xT[: H + F] = x.reshape(NB, H + F).T
    xT[H + F] = 1.0                                        # bias fold row
    xT16 = xT.astype(f16)

    def aug(W, bvec):
        Wa = np.zeros((KPAD, T), f32)
        Wa[: H + F] = np.asarray(W, f32)
        Wa[H + F] = np.asarray(bvec, f32)
        return Wa.astype(f16)

    sW = aug(start_W, start_b)
    eW = aug(end_W, end_b)
    U2 = np.ascontiguousarray(np.asarray(U, f32).reshape(T, C * T)).astype(f16)
    linW = np.asarray(lin_W, f32)
    linWw = np.ascontiguousarray(linW[2 * T :]).astype(f16)
    linPack = np.zeros((128, 5 * C), f32)
    linPack[:, 0:C] = linW[0:128, :]
    linPack[:, C : 2 * C] = linW[128:256, :]
    linPack[:, 2 * C : 3 * C] = linW[T : T + 128, :]
    linPack[:, 3 * C : 4 * C] = linW[T + 128 : 2 * T, :]
    linPack[0, 4 * C : 5 * C] = np.asarray(lin_b, f32)
    linPack16 = linPack.astype(f16)
    wh = np.asarray(width_hidden, f32)

    in_maps = []
    for k in range(NCORES):
        slab = wh[k * SLAB : (k + 1) * SLAB]               # [32, 256, 64]
        whT2 = np.ascontiguousarray(
            slab.transpose(2, 0, 1).reshape(WD, SLAB * S)
        ).astype(f16)
        cols = (
            np.arange(B)[:, None] * S + (k * SLAB + np.arange(SLAB))[None, :]
        ).reshape(-1)
        sxw = np.ascontiguousarray(np.concatenate([sW, xT16[:, cols]], axis=1))
        in_maps.append(
            {
                "sxw": sxw, "eW": eW, "xT": xT16, "U2": U2,
                "whT2": whT2, "linPack": linPack16, "linWw": linWw,
            }
        )

    _CACHE["last_in_maps"] = in_maps
    nc = _get_nc()
    res = run_bass_kernel_spmd(nc, in_maps, core_ids=list(range(NCORES)))

    full = np.empty((B, S, S, C), f32)
    for k in range(NCORES):
        r = res.results[k]["out"].reshape(B, S, C, SLAB)
        full[:, k * SLAB : (k + 1) * SLAB] = r.transpose(0, 3, 1, 2)
    return full
